# revision 46
# baseline (speedup 1.0000x reference)
"""Low-rank multi-head attention Bass kernel for Trainium2 (8 NeuronCores).

Sharding: (batch, query-block) data parallel. 8 cores = 2 batches x 4 query
blocks. No collective: each core receives the full (column-rolled)
feature-major xT of its batch (fp8e4m3, pair-interleaved for DoubleRow
matmuls) and computes full-sequence k1/v1 locally, plus q1 for its own
375-query block.

Key algebraic move: attn @ V is reassociated. Instead of materializing
Vh = v1 @ Wv2h (H,S,64) the kernel accumulates z_h[r,i] = sum_j
v1NT[j,r]*at_h[j,i] (rank-33 per head, j-contraction on the PE via fp8
DoubleRow, 0.5 cycles/row), normalizes by the softmax denominator (z row
32, via the ones column of v1NT), and folds Wv2h into the host-precomputed
out-projection factor A_h = Wv2h @ Wo1_h.T, so o1 = sum_h A_h.T @ znorm_h.

Host-side prep (numpy, inside kernel()):
  - xT rolled/zero-padded to 1536 cols, cast fp8, pair-interleaved
    [5,128,2,1536] so the rank-32 projections run as K=256 DoubleRow.
  - wmT: per-head Wm = Wq2h@Wk2h and all four bias terms folded ([33,H,33]).
  - Wo1v2 = Wv2h @ Wo1_h.T, Wo2Ta with bo_eff = Wo2@Wo1@bv + bo row.
  - ones rows/cols as tiny constants (k1aug/q1Ta/o1a rows, v1NT
    denominator column, zeroed on phantom rows so padding contributes 0).

Engine layout: ACT runs the 120 exp instructions exclusively (~97us busy -
the critical path; exp count B*H*S^2/8 is fixed by softmax attention); PE
does all matmuls (~58us: bf16 scores, fp8-DR z/projections); DVE does PSUM
evacuations + reciprocal + normalize muls; Pool broadcasts 1/denom across
partitions. The flat (pair, j) loop software-pipelines scores two steps
ahead so ACT never starves, v1NT/qh chains fill early PE/DVE slack, and
PSUM is laid out as: scores ring 2x2 banks + z pair 2 + v1NT/qh chain 1 +
o1 accumulator 1 = 8 banks.

Measured (TimelineSim cost model): 134.5us/core vs 219.0us baseline,
rel err 9.5e-3 (gate 2e-2). Offloading exp to DVE via the bf16 bit trick
was tried in four variants and always regressed: the 2-deep scores ring
plus the in-order DVE queue serialize the exp stream across engines.
"""

import sys

sys.path.insert(0, "/opt/trn_rl_repo")

from contextlib import ExitStack

import numpy as np

import concourse.bass as bass
import concourse.tile as tile
from concourse import bacc
from concourse import mybir

F32 = mybir.dt.float32
F32R = mybir.dt.float32r
BF16 = mybir.dt.bfloat16
FP8 = mybir.dt.float8e4
AF = mybir.ActivationFunctionType
DR = mybir.MatmulPerfMode.DoubleRow

H, D, R, N = 20, 64, 32, 1280
NCORES = 8
QP = 4  # query blocks per batch
SCALE = float(D) ** -0.5  # 0.125


def build_nc(S):
    SQ = S // QP          # 375
    SQP = SQ + (SQ % 2)   # 376
    NJ = -(-S // 128)     # 12 j-chunks
    assert NJ % 2 == 0
    NJJ = NJ // 2
    SP = NJ * 128         # 1536 padded sequence
    NPAIR = H // 2

    nc = bacc.Bacc("TRN2", target_bir_lowering=False, debug=False, num_devices=NCORES)

    xT_d = nc.dram_tensor("xT", [5, 128, 2 * SP], FP8, kind="ExternalInput")
    Wcat_d = nc.dram_tensor("Wcat", [128, 3 * 5 * 2 * 32], FP8, kind="ExternalInput")
    wmT_d = nc.dram_tensor("wmT", [33, H * 33], F32R, kind="ExternalInput")
    Wo1v2_d = nc.dram_tensor("Wo1v2", [32, H * 32], BF16, kind="ExternalInput")
    Wo2Ta_d = nc.dram_tensor("Wo2Ta", [33, N], F32R, kind="ExternalInput")
    cbf_d = nc.dram_tensor("cbf", [1, SP], BF16, kind="ExternalInput")
    cf32_d = nc.dram_tensor("cf32", [1, SQP], F32R, kind="ExternalInput")
    czc_d = nc.dram_tensor("czc", [NJ, 128], FP8, kind="ExternalInput")
    out_d = nc.dram_tensor("out", [SQ, N], F32, kind="ExternalOutput")

    def mm(out_, lhsT, rhs, **kw):
        nc.tensor.matmul(out_, lhsT, rhs, **kw)

    ev = [0]

    def evac(dst, src, act_ok=False):
        # PSUM evacuations: DVE, alternating with ACT when ACT has slack
        # (GPSIMD cannot access PSUM on TRN2)
        ev[0] += 1
        if act_ok and ev[0] % 2 == 0:
            nc.scalar.copy(dst, src)
        else:
            nc.vector.tensor_copy(dst, src)

    with tile.TileContext(nc) as tc, ExitStack() as ctx:
        wp = ctx.enter_context(tc.tile_pool(name="wp", bufs=1))
        small_p = ctx.enter_context(tc.tile_pool(name="small_p", bufs=4))
        at2_p = ctx.enter_context(tc.tile_pool(name="at2_p", bufs=4))
        outp = ctx.enter_context(tc.tile_pool(name="outp", bufs=2))

        # ---- persistent SBUF tensors ----
        xTs = [wp.tile([128, 2 * SP], FP8, name=f"xT{c}", tag=f"xT{c}") for c in range(5)]
        Wc = wp.tile([128, 3 * 5 * 2 * 32], FP8)
        Wcv = Wc[:].rearrange("p (w c t r) -> p w c t r", w=3, c=5, t=2)
        wmT = wp.tile([33, H * 33], F32R)
        Wo1v2 = wp.tile([32, H * 32], BF16)
        Wo2Ta = wp.tile([33, N], F32R)
        k1aug = wp.tile([33, SP], BF16)
        q1Ta = wp.tile([33, SQP], F32R)
        qhS = [wp.tile([33, 2 * SQP], BF16, name=f"qh{p}", tag=f"qh{p}") for p in range(NPAIR)]
        # v1NT[jj]: [128 j, 2 (j-parity), 48] fp8; cols 0:32 = v1 rows, col 32
        # = ones (denominator; zero on phantom rows), 33:48 pad (16-aligned
        # half-stride for dual-fp8 ldweights).
        v1NT = [wp.tile([128, 2 * 48], FP8, name=f"v1NT{jj}", tag=f"v1NT{jj}")
                for jj in range(NJJ)]
        zn = [wp.tile([32, 2 * SQP], BF16, name=f"zn{p}", tag=f"zn{p}") for p in range(NPAIR)]
        o1a = wp.tile([33, SQP], F32R)

        # ---- DMA in (tiny constants first so they never gate the
        # k1aug/q1Ta tile dependencies later) ----
        nc.sync.dma_start(Wc[:], Wcat_d[:])
        nc.sync.dma_start(k1aug[32:33, :], cbf_d[:])
        nc.sync.dma_start(q1Ta[32:33, :], cf32_d[:])
        nc.sync.dma_start(o1a[32:33, :], cf32_d[:])
        for c in range(5):
            nc.sync.dma_start(xTs[c][:], xT_d[c, :, :])
        nc.sync.dma_start(wmT[:], wmT_d[:])
        nc.sync.dma_start(Wo1v2[:], Wo1v2_d[:])
        nc.sync.dma_start(Wo2Ta[:], Wo2Ta_d[:])
        for jj in range(NJJ):
            for par in range(2):
                j = 2 * jj + par
                nc.sync.dma_start(
                    v1NT[jj][:].rearrange("p (t r) -> p t r", t=2)[:, par, 32:33],
                    czc_d[j, :].unsqueeze(1),
                )

        # ================= projections =================
        SUBS = [(0, 512), (512, 512), (1024, 512)]

        ctxO = ExitStack()
        psO = ctxO.enter_context(tc.tile_pool(name="psO", bufs=1, space="PSUM"))
        ctxE = ExitStack()
        psE = ctxE.enter_context(tc.tile_pool(name="psE", bufs=1, space="PSUM"))
        o1ps = psO.tile([32, SQP], F32, tag="o1ps")

        # q1 first (its DVE evac gates qh0 while ACT drains the k1 evacs)
        q1ps = psE.tile([32, SQP], F32, tag="q1")
        for c in range(5):
            xv = xTs[c][:].rearrange("p (t s) -> p t s", t=2)
            mm(q1ps[:], Wcv[:, 2, c, :, :], xv[:, :, 0:SQP],
               start=(c == 0), stop=(c == 4), perf_mode=DR)
        nc.vector.tensor_copy(q1Ta[0:32, :], q1ps[:])

        # k1 full sequence: fp8 DoubleRow over feature pairs; evacs on ACT
        k1t = [psE.tile([32, 512], F32, tag=f"pj{s}", name=f"pj{s}") for s in range(3)]
        for c in range(5):
            xv = xTs[c][:].rearrange("p (t s) -> p t s", t=2)
            for s, (s0, sw) in enumerate(SUBS):
                mm(k1t[s][:], Wcv[:, 0, c, :, :], xv[:, :, s0 : s0 + sw],
                   start=(c == 0), stop=(c == 4), perf_mode=DR)
        for s, (s0, sw) in enumerate(SUBS):
            nc.scalar.copy(k1aug[0:32, s0 : s0 + sw], k1t[s][:])

        ctxE.close()

        # ---- attention-phase PSUM: z 2 + v1nt/qh 1 + sc 2x2 + o1 1 = 8 ----
        ctxA = ExitStack()
        z_pool = ctxA.enter_context(tc.tile_pool(name="z_pool", bufs=1, space="PSUM"))
        ps_sc = ctxA.enter_context(tc.tile_pool(name="ps_sc", bufs=2, space="PSUM"))

        def do_qh(hp, act_ok=False):
            for hh in range(2):
                qhp = z_pool.tile([128, 512], F32, tag="v1nt", name=f"qh{hp}{hh}")
                mm(qhp[0:33, 0:SQP],
                   wmT[:].rearrange("p (h m) -> p h m", m=33)[:, 2 * hp + hh, :],
                   q1Ta[:])
                evac(qhS[hp][:, SQP * hh : SQP * hh + SQP], qhp[0:33, 0:SQP],
                     act_ok=act_ok)

        def do_v1nt(j):
            # single-bank chain; runs in PE/DVE slack during early attention
            v1ps = z_pool.tile([128, 512], F32, tag="v1nt", name=f"v1nt{j}")
            for c in range(5):
                xv = xTs[c][:].rearrange("p (t s) -> p t s", t=2)
                mm(v1ps[:, 0:32],
                   xv[:, :, 128 * j : 128 * j + 128],
                   Wcv[:, 1, c, :, :],
                   start=(c == 0), stop=(c == 4), perf_mode=DR)
            evac(v1NT[j // 2][:].rearrange("p (t r) -> p t r", t=2)[:, j % 2, 0:32],
                 v1ps[:, 0:32])

        do_qh(0)  # evacs on DVE; ACT is busy with the k1 evacuations
        do_qh(1)

        # ================= attention core =================
        seq = [(hp, j) for hp in range(NPAIR) for j in range(NJ)]
        scs = {}

        def do_scores(idx):
            hp, j = seq[idx]
            sc = ps_sc.tile([128, 1024], F32, tag="sc", name=f"sc{idx}")
            for hh in range(2):
                mm(sc[:, 512 * hh : 512 * hh + SQP],
                   k1aug[:, 128 * j : 128 * j + 128],
                   qhS[hp][:, SQP * hh : SQP * hh + SQP])
            scs[idx] = sc

        do_scores(0)
        do_scores(1)
        do_scores(2)
        zps = {}
        at2s = {}

        for idx, (hp, j) in enumerate(seq):
            jj, par = j // 2, j % 2
            if j == 0:
                zps[hp] = z_pool.tile([33, 1024], F32, tag="zp", name=f"zp{hp}")
            if par == 0:
                at2s[(hp, jj)] = at2_p.tile(
                    [128, 2 * 2 * SQP], FP8, tag="at2", name=f"at2_{hp}_{jj}"
                )
            sc = scs.pop(idx)
            at2 = at2s[(hp, jj)]
            at2v = at2[:].rearrange("p (t h i) -> p t h i", t=2, h=2)
            nc.scalar.activation(
                at2v[:, par, :, :],
                sc[:].rearrange("p (h i) -> p h i", h=2)[:, :, 0:SQP],
                AF.Exp,
                scale=SCALE,
            )
            if idx + 3 < len(seq):
                do_scores(idx + 3)
            # emit next pair's qh while this pair streams (keeps PSUM bounded)
            if j == 5 and hp + 2 < NPAIR:
                do_qh(hp + 2)
            if hp == 0 and idx < NJ:
                do_v1nt(idx)
            if par == 1:
                at2done = at2s.pop((hp, jj))
                at2dv = at2done[:].rearrange("p (t h i) -> p t h i", t=2, h=2)
                v1v = v1NT[jj][:].rearrange("p (t r) -> p t r", t=2)
                for hh in range(2):
                    mm(zps[hp][:, 512 * hh : 512 * hh + SQP],
                       v1v[:, :, 0:33],
                       at2dv[:, :, hh, :],
                       start=(jj == 0), stop=(jj == NJJ - 1),
                       perf_mode=DR)
            if j == NJ - 1:
                # normalize: znorm_h = z[0:32] * (1/z[32]) and fold into o1
                zp = zps.pop(hp)
                zv = zp[:].rearrange("p (h i) -> p h i", h=2)
                rrs = small_p.tile([1, 2 * SQP], F32R, tag="rrs", name=f"rrs{hp}")
                with nc.allow_low_precision(reason="f32r is bit-identical to f32"):
                    nc.vector.reciprocal(
                        rrs[:].rearrange("p (h i) -> p h i", h=2),
                        zv[32:33, :, 0:SQP],
                    )
                bc_sb = small_p.tile([32, 2 * SQP], F32R, tag="bc_sb", name=f"bcs{hp}")
                nc.gpsimd.partition_broadcast(bc_sb[:], rrs[:])
                for hh in range(2):
                    nc.vector.tensor_mul(
                        zn[hp][:, SQP * hh : SQP * hh + SQP],
                        zv[0:32, hh, 0:SQP],
                        bc_sb[:, SQP * hh : SQP * hh + SQP],
                    )
                for hh in range(2):
                    mm(o1ps[:],
                       Wo1v2[:].rearrange("p (h m) -> p h m", m=32)[:, 2 * hp + hh, :],
                       zn[hp][:, SQP * hh : SQP * hh + SQP],
                       start=(hp == 0 and hh == 0),
                       stop=(hp == NPAIR - 1 and hh == 1))

        # ================= output projection =================
        nc.vector.tensor_copy(o1a[0:32, :], o1ps[:])
        ctxA.close()
        ctxF = ExitStack()
        psF = ctxF.enter_context(tc.tile_pool(name="psF", bufs=4, space="PSUM"))

        ICH = [(i, min(128, SQ - i)) for i in range(0, SQ, 128)]
        OSUB = [(n, min(512, N - n)) for n in range(0, N, 512)]
        for k, (i0, iw) in enumerate(ICH):
            osb = outp.tile([128, N], F32, tag="osb")
            for m, (n0, nw) in enumerate(OSUB):
                fps = psF.tile([128, 512], F32, tag="fps")
                mm(fps[:iw, :nw], o1a[:, i0 : i0 + iw], Wo2Ta[:, n0 : n0 + nw])
                # ACT is idle by now; use it for half the final evacuations
                if (k + m) % 2 == 0:
                    nc.scalar.copy(osb[:iw, n0 : n0 + nw], fps[:iw, :nw])
                else:
                    nc.vector.tensor_copy(osb[:iw, n0 : n0 + nw], fps[:iw, :nw])
                nc.sync.dma_start(out_d[i0 : i0 + iw, n0 : n0 + nw],
                                  osb[:iw, n0 : n0 + nw])
        ctxF.close()
        ctxO.close()

    nc.compile()
    return nc


_NC_CACHE = {}


def _get_nc(S, SQ=None):
    if S not in _NC_CACHE:
        _NC_CACHE[S] = build_nc(S)
    return _NC_CACHE[S]


def _host_prep(inputs, S):
    """Precompute all weight-derived device tensors in numpy."""
    import ml_dtypes

    f = lambda k: np.asarray(inputs[k], dtype=np.float32)
    Wq1, Wq2, bq = f("Wq1"), f("Wq2"), f("bq")
    Wk1, Wk2, bk = f("Wk1"), f("Wk2"), f("bk")
    Wv1, Wv2, bv = f("Wv1"), f("Wv2"), f("bv")
    Wo1, Wo2, bo = f("Wo1"), f("Wo2"), f("bo")

    Wq2h = Wq2.reshape(H, D, R).transpose(0, 2, 1)  # (H,R,D)
    Wk2h = Wk2.reshape(H, D, R)                     # (H,D,R)
    Wm = Wq2h @ Wk2h                                # (H,R,R)
    bqh = bq.reshape(H, 1, D)
    bkh = bk.reshape(H, D, 1)
    b1 = (Wq2h @ bkh)[:, :, 0]                      # (H,R)
    b2 = (bqh @ Wk2h)[:, 0, :]                      # (H,R)
    b3 = (bqh @ bkh)[:, 0, 0]                       # (H,)

    wmT = np.zeros((33, H, 33), np.float32)
    wmT[0:32, :, 0:32] = Wm.transpose(1, 0, 2)
    wmT[32, :, 0:32] = b2
    wmT[0:32, :, 32] = b1.T
    wmT[32, :, 32] = b3

    # A_h = Wv2h @ Wo1_h.T as lhsT[r, (h m)]: o1 += A_h.T @ znorm_h
    Wv2h = Wv2.reshape(H, D, R).transpose(0, 2, 1)  # (H,R,D)
    Wo1h = Wo1.reshape(R, H, D)                     # (m,h,d)
    Wo1v2 = np.einsum("hrd,mhd->rhm", Wv2h, Wo1h)   # (32,H,32)

    # fp8 stationary weights for the DoubleRow projections:
    # Wcat[p, w, cp, t, r] = W[r, 128*(2cp+t)+p] for w in (k1, v1, q1)
    Wcat = np.zeros((128, 3, 5, 2, 32), np.float32)
    for idx, W in enumerate([Wk1, Wv1, Wq1]):
        Wcat[:, idx] = W.T.reshape(5, 2, 128, 32).transpose(2, 0, 1, 3)

    bo_eff = Wo2 @ (Wo1 @ bv) + bo
    Wo2Ta = np.concatenate([Wo2.T, bo_eff[None, :]], axis=0)  # (33,N)

    NJ = -(-S // 128)
    SP = NJ * 128
    SQ = S // QP
    SQP = SQ + (SQ % 2)
    cbf = np.zeros((1, SP), ml_dtypes.bfloat16)
    cbf[0, 0:S] = 1.0
    cf32 = np.ones((1, SQP), np.float32)
    # v1NT denominator column: ones, except zero on phantom rows
    czc = np.zeros((NJ, 128), np.float32)
    for j in range(NJ):
        czc[j, : max(0, min(128, S - 128 * j))] = 1.0

    return {
        "Wcat": np.ascontiguousarray(Wcat.reshape(128, -1).astype(ml_dtypes.float8_e4m3)),
        "wmT": np.ascontiguousarray(wmT.reshape(33, -1)),
        "Wo1v2": np.ascontiguousarray(
            Wo1v2.reshape(32, -1).astype(ml_dtypes.bfloat16)),
        "Wo2Ta": np.ascontiguousarray(Wo2Ta),
        "cbf": cbf,
        "cf32": cf32,
        "czc": czc.astype(ml_dtypes.float8_e4m3),
    }


def kernel(**inputs):
    import ml_dtypes
    from concourse.bass_utils import run_bass_kernel_spmd

    x = np.asarray(inputs["x"], dtype=np.float32)
    B, S, n = x.shape
    assert n == N and B * QP == NCORES
    SQ = S // QP
    NJ = -(-S // 128)
    SP = NJ * 128
    nc = _get_nc(S)

    weights = _host_prep(inputs, S)

    in_maps = []
    for core in range(NCORES):
        b, qi = divmod(core, QP)
        xT = np.roll(x[b].T, -SQ * qi, axis=1)  # (N, S)
        xTp = np.zeros((N, SP), np.float32)
        xTp[:, 0:S] = xT
        # pair-interleaved fp8 tiles: xT8[cp, p, t, s] = xT[128*(2cp+t)+p, s]
        xT8 = np.ascontiguousarray(
            xTp.reshape(5, 2, 128, SP).transpose(0, 2, 1, 3).reshape(5, 128, 2 * SP)
        ).astype(ml_dtypes.float8_e4m3)
        m = {"xT": xT8}
        m.update(weights)
        in_maps.append(m)

    res = run_bass_kernel_spmd(nc, in_maps, core_ids=list(range(NCORES)))
    outs = res.results if hasattr(res, "results") else res

    out = np.zeros((B, S, N), dtype=np.float32)
    for core in range(NCORES):
        b, qi = divmod(core, QP)
        out[b, SQ * qi : SQ * (qi + 1), :] = outs[core]["out"]
    return out
